# revision 2
# baseline (speedup 1.0000x reference)
"""2-layer multi-head GAT on 8 Trainium2 NeuronCores — dst-major rewrite.

Strategy (vs the one-hot-matmul baseline):
  - Nodes degree-sorted; rank-block k (128 nodes) -> core k%8, local block
    k//8. Per-core tables have 50 blocks of 128 rows (49 real + 1 pad row
    block); NTOT = 51200 rows of 256B (bf16[128]).
  - Edge grids are dst-major: gather slot (col=j*G+g, partition=d) holds the
    j-th in-edge of dst d in block g of the superblock. Messages are
    aggregated with strided tree-reductions on DVE — no one-hot matmuls.
  - Gathers: bf16 256B rows, <=1024 idx per dma_gather call, lo/hi windows
    ([0,32768) and [18432,51200)) for int16 indices; edges with src row in
    the overlap are assigned per-dst to balance the two grid widths.
  - Table rows L1: 8x[z_h(8)|1] cols 0..71 | a_src f32x8 at slots 72..87.
    Row L2: h(64) | 1 | pad | t2 f32 at slots 66..67. The |1 columns yield
    softmax denominators inside the same tree reduction.
  - exp on ACT engine; leaky-relu fused as max(0.2x, x) on DVE; attention
    e = asrc[src] + adst[dst] with adst broadcast from per-block stats.
Host does degree sort, window balancing, grid/index construction; device
output is [16, 6272] per core, unpermuted on host.
"""
import numpy as np

import concourse.bass as bass
import concourse.bacc as bacc
import concourse.mybir as mybir
import concourse.tile as tile
import concourse.bass2jax as b2j
from concourse.library_config import mlp
from concourse.tile_rust import add_dep_helper

F32 = mybir.dt.float32
BF16 = mybir.dt.bfloat16
I16 = mybir.dt.int16
I32 = mybir.dt.int32
OP = mybir.AluOpType
AF = mybir.ActivationFunctionType

LEAKY = 0.2
N_CORES = 8
BLK = 128
B = 49
BPAD = 50
SHARD = B * BLK          # 6272
CSTRIDE = BPAD * BLK     # 6400
NTOT = N_CORES * CSTRIDE  # 51200
NREAL = N_CORES * SHARD   # 50176
LO_SIZE = 32768
HI_BASE = NTOT - 32768    # 18432
G = 4
PAD_ASRC = -200.0


def cdiv(a, b):
    return -(-a // b)


def _wrap_idx16(idx):
    n = idx.shape[0]
    w = idx.reshape(n // 16, 16).T.astype(np.int16)
    return np.tile(w, (8, 1))


def _assign_rank(deg_like):
    order = np.argsort(-deg_like, kind="stable")
    rank = np.empty(NREAL, np.int64)
    rank[order] = np.arange(NREAL)
    return order, rank


def _balance(src, dst, rank):
    k = rank // BLK
    core_of = k % N_CORES
    lblk_of = k // N_CORES
    pos_of = rank % BLK
    row_of = core_of * CSTRIDE + lblk_of * BLK + pos_of
    srow = row_of[src]
    ecls = np.where(srow < HI_BASE, 0, np.where(srow < LO_SIZE, 1, 2))
    fl = np.bincount(dst[ecls == 0], minlength=NREAL)
    fr = np.bincount(dst[ecls == 1], minlength=NREAL)
    fh = np.bincount(dst[ecls == 2], minlength=NREAL)
    x = np.clip((fh + fr - fl + 1) // 2, 0, fr)
    nlo = fl + x
    nhi = fh + fr - x
    return row_of, ecls, x, nlo, nhi


def build_layout(edge_indices):
    src = np.asarray(edge_indices[0], np.int64)
    dst = np.asarray(edge_indices[1], np.int64)
    E = src.shape[0]

    deg = np.bincount(dst, minlength=NREAL).astype(np.int64)
    order, rank = _assign_rank(deg)
    row_of, ecls, x, nlo, nhi = _balance(src, dst, rank)
    # iterate: re-sort by width proxy, re-balance
    for _ in range(3):
        proxy = np.maximum(nlo, nhi) * 1000 + deg
        order, rank = _assign_rank(proxy)
        row_of, ecls, x, nlo, nhi = _balance(src, dst, rank)

    k = rank // BLK
    core_of = k % N_CORES
    lblk_of = k // N_CORES
    pos_of = rank % BLK

    # per-edge half + j-slot
    eorder = np.lexsort((np.arange(E), ecls, dst))
    ed, ecl = dst[eorder], ecls[eorder]
    grp_change = np.r_[True, (ed[1:] != ed[:-1]) | (ecl[1:] != ecl[:-1])]
    first = np.maximum.accumulate(np.where(grp_change, np.arange(E), 0))
    j_in_cls = np.arange(E) - first
    half_s = np.where(ecl == 0, 0,
                      np.where(ecl == 2, 1,
                               (j_in_cls >= x[ed]).astype(np.int64)))
    es = src[eorder]
    horder = np.lexsort((j_in_cls, ecl, half_s, ed))
    ed2, es2, eh2 = ed[horder], es[horder], half_s[horder]
    gch = np.r_[True, (ed2[1:] != ed2[:-1]) | (eh2[1:] != eh2[:-1])]
    first2 = np.maximum.accumulate(np.where(gch, np.arange(E), 0))
    j2 = np.arange(E) - first2

    nlo_r = np.zeros(NREAL, np.int64)
    nhi_r = np.zeros(NREAL, np.int64)
    nlo_r[rank] = nlo
    nhi_r[rank] = nhi
    nb = NREAL // BLK
    DLOb = nlo_r.reshape(nb, BLK).max(axis=1)
    DHIb = nhi_r.reshape(nb, BLK).max(axis=1)
    DLO_s = DLOb.reshape(B, N_CORES).max(axis=1)
    DHI_s = DHIb.reshape(B, N_CORES).max(axis=1)

    sb_groups = [list(range(i, min(i + G, B))) for i in range(0, B, G)]
    WLO = np.array([int(DLO_s[g].max()) for g in sb_groups])
    WHI = np.array([int(DHI_s[g].max()) for g in sb_groups])

    dstc = core_of[ed2]
    dstb = lblk_of[ed2]
    dstd = pos_of[ed2]
    srow2 = row_of[es2]

    call_meta = []   # (si, half, W, gsz, idx_offset)
    percore_idx = []
    for c in range(N_CORES):
        m_c = dstc == c
        parts = []
        off = 0
        for si, grp in enumerate(sb_groups):
            gsz = len(grp)
            for hv, W, base in ((0, int(WLO[si]), 0),
                                (1, int(WHI[si]), HI_BASE)):
                if W == 0:
                    continue
                grid = np.zeros((W, gsz, BLK), np.int64)
                padbase = 6272 if hv == 0 else 7 * CSTRIDE + 6272 - HI_BASE
                grid[:, :, :] = padbase + np.arange(BLK)[None, None, :]
                m = m_c & (eh2 == hv) & np.isin(dstb, grp)
                grid[j2[m], dstb[m] - grp[0], dstd[m]] = srow2[m] - base
                assert grid.min() >= 0 and grid.max() < 32768
                parts.append(grid.reshape(-1))
                if c == 0:
                    call_meta.append((si, hv, W, gsz, off))
                off += W * gsz * BLK
        percore_idx.append(np.concatenate(parts).astype(np.int16))

    tot_idx = len(percore_idx[0])
    assert all(len(p) == tot_idx for p in percore_idx)
    assert tot_idx % 16 == 0

    return dict(order=order, rank=rank, sb_groups=sb_groups,
                WLO=WLO, WHI=WHI, call_meta=call_meta,
                percore_idx=percore_idx, tot_idx=tot_idx)


def host_prep(x, edge_indices, W1, a_src1, a_dst1, b1, W2, a_src2, a_dst2,
              b2):
    N, Din = x.shape
    D1 = W1.shape[1]
    H1, Dh1 = a_src1.shape
    D2 = W2.shape[1]
    assert Din == 128 and D1 == 64 and H1 == 8 and D2 == 16

    lay = build_layout(edge_indices)
    order = lay["order"]

    A_src = np.zeros((D1, H1), np.float64)
    A_dst = np.zeros((D1, H1), np.float64)
    for h in range(H1):
        A_src[h * Dh1:(h + 1) * Dh1, h] = a_src1[h]
        A_dst[h * Dh1:(h + 1) * Dh1, h] = a_dst1[h]
    W1f = np.concatenate([W1.astype(np.float64),
                          W1.astype(np.float64) @ A_src,
                          W1.astype(np.float64) @ A_dst],
                         axis=1).astype(np.float32)
    w_as2 = (W2.astype(np.float64) @ a_src2[0].astype(np.float64))
    w_ad2 = (W2.astype(np.float64) @ a_dst2[0].astype(np.float64))

    xpad = np.zeros((NREAL, Din), np.float32)
    xpad[:N] = x

    import ml_dtypes
    padrow_u16 = np.zeros((BLK, 128), np.uint16)
    pr_f32 = np.full(8, PAD_ASRC, np.float32).view(np.uint16)
    padrow_u16[:, 72:88] = pr_f32[None, :]
    padrow_u16[:, 66:68] = np.full(1, PAD_ASRC,
                                   np.float32).view(np.uint16)[None, :]
    padrow = padrow_u16.view(ml_dtypes.bfloat16)

    consts = dict(
        W1f=W1f,
        b1r=np.tile(b1[None, :], (BLK, 1)).astype(np.float32),
        wa2r=np.tile(w_as2[None, :], (BLK, 1)).astype(np.float32),
        wd2r=np.tile(w_ad2[None, :], (BLK, 1)).astype(np.float32),
        W2=W2.astype(np.float32),
        b2c=b2.reshape(D2, 1).astype(np.float32),
        padrow=padrow,
    )

    in_maps = []
    for c in range(N_CORES):
        ranks = ((np.arange(B * BLK) // BLK) * N_CORES + c) * BLK \
            + (np.arange(B * BLK) % BLK)
        nodes = order[ranks]
        m = dict(consts)
        m["xTs"] = np.ascontiguousarray(xpad[nodes].T)
        m["idx16"] = _wrap_idx16(lay["percore_idx"][c])
        in_maps.append(m)

    cfg = dict(lay=lay, N=N, D2=D2)
    return cfg, in_maps


def _tree_reduce(nc, p, cur, W, width, tagp, maxW):
    """Pairwise tree-sum over the leading (j) dim of tile [128, W, G, width].
    Returns the final tile [128, 1(, G), width]."""
    n = W
    si = 0
    nmax = maxW
    while n > 1:
        h = (n + 1) // 2
        hmax = (nmax + 1) // 2
        dt_out = F32 if h <= 2 else BF16
        nxt = p.tile([128, hmax, G, width], dt_out, tag=f"{tagp}{si}")
        ps = cur[:].ap[0][0]
        row = G * width
        a0 = bass.AP(cur.tensor, cur[:].offset,
                     [[ps, 128], [row, n - h], [1, row]])
        a1 = bass.AP(cur.tensor, cur[:].offset + h * row,
                     [[ps, 128], [row, n - h], [1, row]])
        nc.vector.tensor_tensor(nxt[:, 0:n - h], a0, a1, op=OP.add)
        if h > n - h:
            keep = bass.AP(cur.tensor, cur[:].offset + (n - h) * row,
                           [[ps, 128], [row, 2 * h - n], [1, row]])
            nc.vector.tensor_copy(nxt[:, n - h:h], keep)
        cur = nxt
        n = h
        nmax = hmax
        si += 1
    return cur


def build_nc(cfg, repeat=1):
    lay = cfg["lay"]
    sb_groups = lay["sb_groups"]
    WLO, WHI = lay["WLO"], lay["WHI"]
    call_meta = lay["call_meta"]
    TOT = lay["tot_idx"]
    D2 = cfg["D2"]

    calls = {}
    for (si, hv, W, gsz, off) in call_meta:
        calls[(si, hv)] = (W, gsz, off)

    MAXC = max(W * gsz for (W, gsz, off) in calls.values())
    MAXW_LO = int(WLO.max())
    MAXW_HI = int(WHI.max())
    MAXW = max(MAXW_LO, MAXW_HI)

    nc = bacc.Bacc("TRN2", target_bir_lowering=False, debug=False,
                   num_devices=N_CORES, num_swdge_queues=4)

    din = {}
    for name, shape, dt in [
            ("xTs", [128, SHARD], F32), ("W1f", [128, 80], F32),
            ("b1r", [128, 64], F32), ("wa2r", [128, 64], F32),
            ("wd2r", [128, 64], F32), ("W2", [64, D2], F32),
            ("b2c", [D2, 1], F32), ("idx16", [128, TOT // 16], I16),
            ("padrow", [BLK, 128], BF16)]:
        din[name] = nc.dram_tensor(name, shape, dt, kind="ExternalInput").ap()

    z1shard = nc.dram_tensor("z1shard", [CSTRIDE, 128], BF16).ap()
    z1full = nc.dram_tensor("z1full", [NTOT, 128], BF16,
                            addr_space="Shared").ap()
    h2shard = nc.dram_tensor("h2shard", [CSTRIDE, 128], BF16).ap()
    h2full = nc.dram_tensor("h2full", [NTOT, 128], BF16,
                            addr_space="Shared").ap()
    out2T = nc.dram_tensor("out2T", [D2, SHARD], F32,
                           kind="ExternalOutput").ap()
    import os
    dbg = os.environ.get("KNEW_DEBUG", "") in ("1", "2")
    dbg2 = os.environ.get("KNEW_DEBUG", "") == "2"
    if dbg:
        dbgz = nc.dram_tensor("dbgz", [CSTRIDE, 128], BF16,
                              kind="ExternalOutput").ap()
        dbgh = nc.dram_tensor("dbgh", [CSTRIDE, 128], BF16,
                              kind="ExternalOutput").ap()
    if dbg2:
        W0 = calls[(0, 0)][0]
        dbg_zg = nc.dram_tensor("dbg_zg", [128, W0 * G * 128], BF16,
                                kind="ExternalOutput").ap()
        dbg_e = nc.dram_tensor("dbg_e", [128, W0 * G * 8], F32,
                               kind="ExternalOutput").ap()
        dbg_w = nc.dram_tensor("dbg_w", [128, W0 * G * 8], BF16,
                               kind="ExternalOutput").ap()
        dbg_msg = nc.dram_tensor("dbg_msg", [128, W0 * G * 72], BF16,
                                 kind="ExternalOutput").ap()
        dbg_S = nc.dram_tensor("dbg_S", [128, G * 72], F32,
                               kind="ExternalOutput").ap()

    from contextlib import ExitStack
    with tile.TileContext(nc) as tc, ExitStack() as top:
        nc.gpsimd.load_library(mlp)
        cp = top.enter_context(tc.tile_pool(name="consts", bufs=1))

        xts = cp.tile([128, SHARD], F32)
        w1f = cp.tile([128, 80], F32)
        b1r = cp.tile([128, 64], F32)
        wa2r = cp.tile([128, 64], F32)
        wd2r = cp.tile([128, 64], F32)
        w2 = cp.tile([64, D2], F32)
        b2c = cp.tile([D2, 1], F32)
        idxs = cp.tile([128, TOT // 16], I16)
        for t, name in [(xts, "xTs"), (w1f, "W1f"), (b1r, "b1r"),
                        (wa2r, "wa2r"), (wd2r, "wd2r"), (w2, "W2"),
                        (b2c, "b2c"), (idxs, "idx16")]:
            nc.sync.dma_start(t[:], din[name][:])

        iota_i = cp.tile([128, 128], I32)
        iota_c = cp.tile([128, 1], I32)
        iota = cp.tile([128, 128], F32)
        iotac = cp.tile([128, 1], F32)
        ident = cp.tile([128, 128], F32)
        ones8 = cp.tile([128, 8], F32)
        nc.gpsimd.iota(iota_i[:], [[1, 128]], base=0, channel_multiplier=0)
        nc.gpsimd.iota(iota_c[:], [[1, 1]], base=0, channel_multiplier=1)
        nc.vector.tensor_copy(iota[:], iota_i[:])
        nc.vector.tensor_copy(iotac[:], iota_c[:])
        nc.vector.tensor_scalar(ident[:], iota[:], iotac[:, :1], None,
                                op0=OP.is_equal)
        nc.vector.memset(ones8[:], 1.0)

        adstp = cp.tile([128, B, 8], F32)
        adst2p = cp.tile([128, B], F32)

        qn = [0]

        def gather_cols(p, tag, table, ncols, idx_off, ag_inst, maxc):
            zg = p.tile([128, maxc, 128], BF16, tag=tag)
            c0 = 0
            while c0 < ncols:
                take = min(8, ncols - c0)
                g = nc.gpsimd.dma_gather(
                    zg[:, c0:c0 + take, :], table,
                    idxs[:, (idx_off + c0 * 128) // 16:
                         (idx_off + (c0 + take) * 128) // 16],
                    take * 128, take * 128, 128,
                    single_packet=True, queue_num=qn[0] % 4)
                qn[0] += 1
                if ag_inst is not None:
                    add_dep_helper(g.ins, ag_inst.ins, sync=True,
                                   reason="gather after allgather")
                c0 += take
            return zg

        for _rep in range(repeat):
            # ---------- phase 1 ----------
            z1w = []
            with tc.tile_pool(name="p1", bufs=3) as p1, \
                 tc.tile_pool(name="p1ps", bufs=2, space="PSUM") as p1ps:
                for b in range(B):
                    pz = p1ps.tile([128, 80], F32, space="PSUM", tag="pz")
                    nc.tensor.matmul(pz[:], xts[:, b * 128:(b + 1) * 128],
                                     w1f[:], start=True, stop=True)
                    nc.vector.tensor_copy(adstp[:, b, :], pz[:, 72:80])
                    row = p1.tile([128, 128], BF16, tag="row")
                    zi_dst = bass.AP(row.tensor, row[:].offset,
                                     [[row[:].ap[0][0], 128], [9, 8], [1, 8]])
                    zi_src = bass.AP(pz.tensor, pz[:].offset,
                                     [[pz[:].ap[0][0], 128], [8, 8], [1, 8]])
                    nc.vector.tensor_copy(zi_dst, zi_src)
                    on_dst = bass.AP(row.tensor, row[:].offset + 8,
                                     [[row[:].ap[0][0], 128], [9, 8]])
                    nc.vector.tensor_copy(on_dst, ones8[:])
                    rf = row[:].bitcast(F32)
                    as_dst = bass.AP(rf.tensor, rf.offset + 36,
                                     [[rf.ap[0][0], 128], [1, 8]])
                    nc.vector.tensor_copy(as_dst, pz[:, 64:72])
                    w = nc.sync.dma_start(
                        z1shard[b * 128:(b + 1) * 128, :], row[:])
                    z1w.append(w)
                w = nc.sync.dma_start(z1shard[SHARD:CSTRIDE, :],
                                      din["padrow"][:])
                z1w.append(w)

            if dbg:
                nc.sync.dma_start(dbgz[:], z1shard[:])
            ag1 = nc.gpsimd.collective_compute(
                "AllGather", OP.bypass,
                replica_groups=[list(range(N_CORES))],
                ins=[z1shard[:]], outs=[z1full[:]])
            for w in z1w:
                add_dep_helper(ag1.ins, w.ins, sync=True,
                               reason="z1 writes before ag1")

            # ---------- layer 1 edge phase ----------
            h2w = []
            with tc.tile_pool(name="l1", bufs=2) as p, \
                 tc.tile_pool(name="l1s", bufs=1) as p1b:
                for si, grp in enumerate(sb_groups):
                    gsz = len(grp)
                    b0 = grp[0]
                    halves = []
                    for hv, table in ((0, z1full[0:LO_SIZE, :]),
                                      (1, z1full[HI_BASE:NTOT, :])):
                        if (si, hv) not in calls:
                            continue
                        W, gsz_c, off = calls[(si, hv)]
                        assert gsz_c == gsz
                        zg = gather_cols(p, "zg", table, W * gsz, off,
                                         ag1, MAXC)
                        zps = zg[:].ap[0][0]
                        zf = zg[:].bitcast(F32)
                        maxw = MAXW
                        e = p.tile([128, maxw, G, 8], F32, tag="e")
                        asrc_v = bass.AP(
                            zf.tensor, zf.offset + 36,
                            [[zf.ap[0][0], 128], [64 * gsz, W], [64, gsz],
                             [1, 8]])
                        adst_bc = bass.AP(
                            adstp.tensor, adstp[:].offset + b0 * 8,
                            [[adstp[:].ap[0][0], 128], [0, W], [8, gsz],
                             [1, 8]])
                        nc.vector.tensor_tensor(e[:, 0:W, 0:gsz], asrc_v,
                                                adst_bc, op=OP.add)
                        lr = p.tile([128, maxw, G, 8], F32, tag="lr")
                        nc.vector.scalar_tensor_tensor(
                            lr[:, 0:W, 0:gsz], e[:, 0:W, 0:gsz], LEAKY,
                            e[:, 0:W, 0:gsz], op0=OP.mult, op1=OP.max)
                        wt = p.tile([128, maxw, G, 8], BF16, tag="w")
                        nc.scalar.activation(wt[:, 0:W, 0:gsz],
                                             lr[:, 0:W, 0:gsz], AF.Exp)
                        msg = p.tile([128, maxw, G, 72], BF16,
                                     tag="msg")
                        for g in range(gsz):
                            w_bc = bass.AP(
                                wt.tensor, wt[:].offset + g * 8,
                                [[wt[:].ap[0][0], 128], [G * 8, W], [1, 8],
                                 [0, 9]])
                            zsl = bass.AP(
                                zg.tensor, zg[:].offset + g * 128,
                                [[zps, 128], [gsz * 128, W], [1, 72]])
                            mo = bass.AP(
                                msg.tensor, msg[:].offset + g * 72,
                                [[msg[:].ap[0][0], 128], [G * 72, W],
                                 [1, 72]])
                            nc.vector.tensor_tensor(mo, zsl, w_bc, op=OP.mult)
                        if dbg2 and _rep == 0 and si == 0 and hv == 0:
                            nc.sync.dma_start(dbg_zg[:], bass.AP(
                                zg.tensor, zg[:].offset,
                                [[zps, 128], [1, W * G * 128]]))
                            nc.sync.dma_start(dbg_e[:], bass.AP(
                                e.tensor, e[:].offset,
                                [[e[:].ap[0][0], 128], [1, W * G * 8]]))
                            nc.sync.dma_start(dbg_w[:], bass.AP(
                                wt.tensor, wt[:].offset,
                                [[wt[:].ap[0][0], 128], [1, W * G * 8]]))
                            nc.sync.dma_start(dbg_msg[:], bass.AP(
                                msg.tensor, msg[:].offset,
                                [[msg[:].ap[0][0], 128], [1, W * G * 72]]))
                        red = _tree_reduce(nc, p, msg, W, 72,
                                           "t_", MAXW)
                        if not halves:
                            Sh = p1b.tile([128, G, 72], F32, tag="Sh")
                            nc.vector.tensor_copy(Sh[:, 0:gsz],
                                                  red[:, 0, 0:gsz])
                            halves.append(Sh)
                        else:
                            halves.append(red)
                    S = p1b.tile([128, G, 72], F32, tag="S")
                    if len(halves) == 2:
                        nc.vector.tensor_tensor(S[:, 0:gsz],
                                                halves[0][:, 0:gsz],
                                                halves[1][:, 0, 0:gsz],
                                                op=OP.add)
                    else:
                        nc.vector.tensor_copy(S[:, 0:gsz],
                                              halves[0][:, 0:gsz])
                    sps = S[:].ap[0][0]
                    den = bass.AP(S.tensor, S[:].offset + 8,
                                  [[sps, 128], [72, gsz], [9, 8]])
                    denp = p1b.tile([128, G, 8], F32, tag="denp")
                    nc.vector.tensor_scalar(denp[:, 0:gsz], den, 1e-16, None,
                                            op0=OP.add)
                    rden = p1b.tile([128, G, 8], F32, tag="rden")
                    nc.vector.reciprocal(rden[:, 0:gsz], denp[:, 0:gsz])
                    num = bass.AP(S.tensor, S[:].offset,
                                  [[sps, 128], [72, gsz], [9, 8], [1, 8]])
                    rd_bc = bass.AP(rden.tensor, rden[:].offset,
                                    [[rden[:].ap[0][0], 128], [8, gsz],
                                     [1, 8], [0, 8]])
                    o1 = p1b.tile([128, G, 64], F32, tag="o1")
                    o1v = bass.AP(o1.tensor, o1[:].offset,
                                  [[o1[:].ap[0][0], 128], [64, gsz], [8, 8],
                                   [1, 8]])
                    nc.vector.tensor_tensor(o1v, num, rd_bc, op=OP.mult)
                    if dbg2 and _rep == 0 and si == 0:
                        nc.sync.dma_start(dbg_S[:], bass.AP(
                            S.tensor, S[:].offset,
                            [[S[:].ap[0][0], 128], [1, G * 72]]))
                    x1 = p1b.tile([128, G, 64], F32, tag="x1")
                    b1_bc = bass.AP(b1r.tensor, b1r[:].offset,
                                    [[b1r[:].ap[0][0], 128], [0, gsz],
                                     [1, 64]])
                    nc.vector.tensor_tensor(x1[:, 0:gsz], o1[:, 0:gsz],
                                            b1_bc, op=OP.add)
                    xm = p1b.tile([128, G, 64], F32, tag="xm")
                    nc.vector.tensor_scalar(xm[:, 0:gsz], x1[:, 0:gsz], 0.0,
                                            None, op0=OP.min)
                    u = p1b.tile([128, G, 64], F32, tag="u")
                    nc.scalar.activation(u[:, 0:gsz], xm[:, 0:gsz], AF.Exp)
                    v = p1b.tile([128, G, 64], F32, tag="v")
                    nc.vector.tensor_scalar(v[:, 0:gsz], x1[:, 0:gsz], 0.0,
                                            None, op0=OP.max)
                    hh = p1b.tile([128, G, 64], F32, tag="hh")
                    nc.vector.scalar_tensor_tensor(
                        hh[:, 0:gsz], u[:, 0:gsz], -1.0, v[:, 0:gsz],
                        op0=OP.add, op1=OP.add)
                    row2 = p1b.tile([128, G, 128], BF16, tag="row2")
                    r2c = bass.AP(row2.tensor, row2[:].offset,
                                  [[row2[:].ap[0][0], 128], [128, gsz],
                                   [1, 64]])
                    nc.vector.tensor_copy(r2c, hh[:, 0:gsz])
                    on2 = bass.AP(row2.tensor, row2[:].offset + 64,
                                  [[row2[:].ap[0][0], 128], [128, gsz]])
                    nc.vector.memset(on2, 1.0)
                    r2f = row2[:].bitcast(F32)
                    tr = p1b.tile([128, 64], F32, tag="tr")
                    for g in range(gsz):
                        t2v = bass.AP(r2f.tensor, r2f.offset + g * 64 + 33,
                                      [[r2f.ap[0][0], 128], [1, 1]])
                        nc.vector.scalar_tensor_tensor(
                            tr[:], hh[:, g], 1.0, wa2r[:],
                            op0=OP.mult, op1=OP.mult, accum_out=t2v)
                        nc.vector.scalar_tensor_tensor(
                            tr[:], hh[:, g], 1.0, wd2r[:],
                            op0=OP.mult, op1=OP.mult,
                            accum_out=adst2p[:, b0 + g:b0 + g + 1])
                    h2_dst = bass.AP(
                        h2shard.tensor,
                        h2shard[:].offset + b0 * 128 * 128,
                        [[128, 128], [128 * 128, gsz], [1, 128]])
                    w = nc.sync.dma_start(
                        h2_dst,
                        bass.AP(row2.tensor, row2[:].offset,
                                [[row2[:].ap[0][0], 128], [128, gsz],
                                 [1, 128]]))
                    h2w.append(w)
                w = nc.sync.dma_start(h2shard[SHARD:CSTRIDE, :],
                                      din["padrow"][:])
                h2w.append(w)

            if dbg:
                nc.sync.dma_start(dbgh[:], h2shard[:])
            ag2 = nc.gpsimd.collective_compute(
                "AllGather", OP.bypass,
                replica_groups=[list(range(N_CORES))],
                ins=[h2shard[:]], outs=[h2full[:]])
            for w in h2w:
                add_dep_helper(ag2.ins, w.ins, sync=True,
                               reason="h2 writes before ag2")

            # ---------- layer 2 edge phase ----------
            with tc.tile_pool(name="l2", bufs=2) as p, \
                 tc.tile_pool(name="l2s", bufs=1) as p2b, \
                 tc.tile_pool(name="l2ps", bufs=2, space="PSUM") as pps:
                for si, grp in enumerate(sb_groups):
                    gsz = len(grp)
                    b0 = grp[0]
                    halves = []
                    for hv, table in ((0, h2full[0:LO_SIZE, :]),
                                      (1, h2full[HI_BASE:NTOT, :])):
                        if (si, hv) not in calls:
                            continue
                        W, gsz_c, off = calls[(si, hv)]
                        zg = gather_cols(p, "zg", table, W * gsz, off,
                                         ag2, MAXC)
                        zps = zg[:].ap[0][0]
                        zf = zg[:].bitcast(F32)
                        maxw = MAXW
                        e = p.tile([128, maxw, G], F32, tag="e2")
                        t2_v = bass.AP(
                            zf.tensor, zf.offset + 33,
                            [[zf.ap[0][0], 128], [64 * gsz, W], [64, gsz]])
                        ad_bc = bass.AP(
                            adst2p.tensor, adst2p[:].offset + b0,
                            [[adst2p[:].ap[0][0], 128], [0, W], [1, gsz]])
                        nc.vector.tensor_tensor(e[:, 0:W, 0:gsz], t2_v,
                                                ad_bc, op=OP.add)
                        lr = p.tile([128, maxw, G], F32, tag="lr2")
                        nc.vector.scalar_tensor_tensor(
                            lr[:, 0:W, 0:gsz], e[:, 0:W, 0:gsz], LEAKY,
                            e[:, 0:W, 0:gsz], op0=OP.mult, op1=OP.max)
                        wt = p.tile([128, maxw, G], BF16, tag="w2")
                        nc.scalar.activation(wt[:, 0:W, 0:gsz],
                                             lr[:, 0:W, 0:gsz], AF.Exp)
                        msg = p.tile([128, maxw, G, 65], BF16,
                                     tag="msg2")
                        for g in range(gsz):
                            w_bc = bass.AP(
                                wt.tensor, wt[:].offset + g,
                                [[wt[:].ap[0][0], 128], [G, W], [0, 65]])
                            zsl = bass.AP(
                                zg.tensor, zg[:].offset + g * 128,
                                [[zps, 128], [gsz * 128, W], [1, 65]])
                            mo = bass.AP(
                                msg.tensor, msg[:].offset + g * 65,
                                [[msg[:].ap[0][0], 128], [G * 65, W],
                                 [1, 65]])
                            nc.vector.tensor_tensor(mo, zsl, w_bc, op=OP.mult)
                        red = _tree_reduce(nc, p, msg, W, 65,
                                           "u_", MAXW)
                        if not halves:
                            Sh = p2b.tile([128, G, 65], F32, tag="Sh2")
                            nc.vector.tensor_copy(Sh[:, 0:gsz],
                                                  red[:, 0, 0:gsz])
                            halves.append(Sh)
                        else:
                            halves.append(red)
                    S = p2b.tile([128, G, 65], F32, tag="S2")
                    if len(halves) == 2:
                        nc.vector.tensor_tensor(S[:, 0:gsz],
                                                halves[0][:, 0:gsz],
                                                halves[1][:, 0, 0:gsz],
                                                op=OP.add)
                    else:
                        nc.vector.tensor_copy(S[:, 0:gsz],
                                              halves[0][:, 0:gsz])
                    sps = S[:].ap[0][0]
                    den = bass.AP(S.tensor, S[:].offset + 64,
                                  [[sps, 128], [65, gsz]])
                    denp = p2b.tile([128, G], F32, tag="denp2")
                    nc.vector.tensor_scalar(denp[:, 0:gsz], den, 1e-16, None,
                                            op0=OP.add)
                    rden = p2b.tile([128, G], F32, tag="rden2")
                    nc.vector.reciprocal(rden[:, 0:gsz], denp[:, 0:gsz])
                    num = bass.AP(S.tensor, S[:].offset,
                                  [[sps, 128], [65, gsz], [1, 64]])
                    rd_bc = bass.AP(rden.tensor, rden[:].offset,
                                    [[rden[:].ap[0][0], 128], [1, gsz],
                                     [0, 64]])
                    agg = p2b.tile([128, G, 64], F32, tag="agg")
                    nc.vector.tensor_tensor(agg[:, 0:gsz], num, rd_bc,
                                            op=OP.mult)
                    for g in range(gsz):
                        ptr = pps.tile([64, 128], F32, space="PSUM",
                                       tag="ptr")
                        nc.tensor.transpose(ptr[:], agg[:, g], ident[:])
                        aggT = p2b.tile([64, 128], F32, tag="aggT")
                        nc.vector.tensor_copy(aggT[:], ptr[:])
                        po2 = pps.tile([D2, 128], F32, space="PSUM",
                                       tag="po2")
                        nc.tensor.matmul(po2[:], w2[:], aggT[:], start=True,
                                         stop=True)
                        o2 = p2b.tile([D2, 128], F32, tag="o2")
                        nc.scalar.activation(o2[:], po2[:], AF.Identity,
                                             bias=b2c[:, :1])
                        nc.sync.dma_start(
                            out2T[:, (b0 + g) * 128:(b0 + g + 1) * 128],
                            o2[:])

    nc.compile()
    return nc


class CachedRunner:
    def __init__(self, nc, n_cores):
        import jax
        from jax.sharding import Mesh, PartitionSpec, NamedSharding
        from jax.experimental.shard_map import shard_map
        b2j.install_neuronx_cc_hook()
        self.nc = nc
        self.n_cores = n_cores
        in_names, out_names, out_avals = [], [], []
        for alloc in nc.m.functions[0].allocations:
            if not isinstance(alloc, mybir.MemoryLocationSet):
                continue
            name = alloc.memorylocations[0].name
            if alloc.kind == "ExternalInput":
                if (nc.partition_id_tensor is None
                        or name != nc.partition_id_tensor.name):
                    in_names.append(name)
            elif alloc.kind == "ExternalOutput":
                out_names.append(name)
                out_avals.append(jax.core.ShapedArray(
                    tuple(alloc.tensor_shape), mybir.dt.np(alloc.dtype)))
        self.in_names, self.out_names, self.out_avals = \
            in_names, out_names, out_avals
        n_params = len(in_names)
        all_in = list(in_names) + list(out_names)
        if nc.partition_id_tensor is not None:
            all_in.append(nc.partition_id_tensor.name)

        def _body(*args):
            operands = list(args)
            if nc.partition_id_tensor is not None:
                operands.append(b2j.partition_id_tensor())
            outs = b2j._bass_exec_p.bind(
                *operands, out_avals=tuple(out_avals), in_names=tuple(all_in),
                out_names=tuple(out_names), lowering_input_output_aliases=(),
                sim_require_finite=False, sim_require_nnan=False, nc=nc)
            return tuple(outs)

        self.jax = jax
        self.devices = jax.devices()[:n_cores]
        self.mesh = Mesh(np.asarray(self.devices), ("core",))
        donate = tuple(range(n_params, n_params + len(out_names)))
        self.fn = jax.jit(
            shard_map(_body, mesh=self.mesh,
                      in_specs=(PartitionSpec("core"),) * (n_params +
                                                           len(out_names)),
                      out_specs=(PartitionSpec("core"),) * len(out_names),
                      check_rep=False),
            donate_argnums=donate, keep_unused=True)
        self.sh = NamedSharding(self.mesh, PartitionSpec("core"))
        self.dev_ins = None

    def put_inputs(self, in_maps):
        concat = [np.concatenate([np.asarray(in_maps[c][n])
                                  for c in range(self.n_cores)], axis=0)
                  for n in self.in_names]
        self.dev_ins = [self.jax.device_put(a, self.sh) for a in concat]
        for a in self.dev_ins:
            a.block_until_ready()

    def __call__(self):
        jnp = self.jax.numpy
        zeros = [self.jax.device_put(
            jnp.zeros((self.n_cores * av.shape[0], *av.shape[1:]), av.dtype),
            self.sh) for av in self.out_avals]
        outs = self.fn(*self.dev_ins, *zeros)
        return {name: np.asarray(outs[i]).reshape(
                    self.n_cores, *self.out_avals[i].shape)
                for i, name in enumerate(self.out_names)}


_STATE = {}


def _fingerprint(inputs):
    import hashlib
    h = hashlib.sha256()
    for k in sorted(inputs):
        a = np.asarray(inputs[k])
        h.update(k.encode())
        h.update(str(a.shape).encode())
        h.update(str(a.dtype).encode())
        h.update(np.ascontiguousarray(a).tobytes())
    return h.hexdigest()


def _get_state(inputs, repeat=1):
    key = (_fingerprint(inputs), repeat)
    st = _STATE.get("st")
    if st is not None and st["key"] == key:
        return st
    cfg, in_maps = host_prep(
        np.asarray(inputs["x"], np.float32),
        np.asarray(inputs["edge_indices"]),
        np.asarray(inputs["W1"], np.float32),
        np.asarray(inputs["a_src1"], np.float32),
        np.asarray(inputs["a_dst1"], np.float32),
        np.asarray(inputs["b1"], np.float32),
        np.asarray(inputs["W2"], np.float32),
        np.asarray(inputs["a_src2"], np.float32),
        np.asarray(inputs["a_dst2"], np.float32),
        np.asarray(inputs["b2"], np.float32))
    nc = build_nc(cfg, repeat=repeat)
    runner = CachedRunner(nc, N_CORES)
    runner.put_inputs(in_maps)
    st = {"key": key, "cfg": cfg, "runner": runner}
    _STATE["st"] = st
    return st


def kernel(**inputs):
    st = _get_state(inputs)
    res = st["runner"]()
    cfg = st["cfg"]
    order = cfg["lay"]["order"]
    # out2T: [core, 16, SHARD]; node at rank r: core=(r//128)%8,
    # lblk=(r//128)//8, pos=r%128 -> col lblk*128+pos
    o = res["out2T"]
    ranks = np.arange(NREAL)
    cores = (ranks // BLK) % N_CORES
    cols = ((ranks // BLK) // N_CORES) * BLK + (ranks % BLK)
    vals = o[cores, :, cols]          # [NREAL, 16]
    out = np.zeros((NREAL, cfg["D2"]), np.float32)
    out[order[ranks]] = vals
    return np.ascontiguousarray(out[:cfg["N"]].astype(np.float32))


def measure_device_time_ns(inputs, k0=1, k1=9, reps=25):
    import time
    times = {}
    saved = _STATE.pop("st", None)
    for K in (k0, k1):
        st = _get_state(inputs, repeat=K)
        st["runner"]()
        st["runner"]()
        best = 1e9
        for _ in range(reps):
            t0 = time.time()
            st["runner"]()
            best = min(best, time.time() - t0)
        times[K] = best
        _STATE.pop("st", None)
    if saved is not None:
        _STATE["st"] = saved
    return (times[k1] - times[k0]) / (k1 - k0) * 1e9
